# revision 1
# baseline (speedup 1.0000x reference)
"""Trainium2 Bass kernel for nn_Char_30322469110372 (retrieval_knn).

Reference computation (per query b):
  ce   = row-normalized ce_raw (+ zero pad row for index -1)
  q    = ce[qidx[b]]
  for side in (l, r):
    u_side      = W_side @ q                     # [C]
    score[k]    = ce[ixs_c[b,k]] . u_side        # masked to -1e30 where ixs==-1
    attn        = softmax(score)
    emb_side    = sum_k attn[k] * wvec[ixs_w[b,k]]
  gate = softmax([attn_l, attn_r] @ gL_w.T + gL_b)
  out  = gate[0]*emb_l + gate[1]*emb_r

Sharding: data-parallel over B across 8 cores; ce/wvec tables replicated.

Device algorithm per core (B_core=2048 queries, 16 tiles of 128 queries,
processed in chunks of 4 tiles so gather DMA, DVE, ACT and Pool overlap):
  - normalization folded into scores: score = (ctx_raw.u_raw) * rctx * rq with
    rctx/rq = 1/max(||row||,1e-12) computed on gathered rows only.
  - scores are bounded (|score| <= ~1.2) so softmax needs no max-shift;
    exp(-1e30) underflows to exactly 0 for pad slots.
  - per tile: indirect-DMA gather of 18 ce rows/query; PE transposes q and
    computes u = [qT]^T @ [lW^T | rW^T]; DVE does the 17 dot products; ACT
    squares rows, DVE reduces to row sum-squares.
  - per chunk: softmax + gate pipeline on [128, 4*17] staging buffers.
  - per tile: gather 17 wvec rows/query (pad slots clamped to row 0; their
    softmax weight is exactly 0), then a 17-step scalar*tensor+tensor
    accumulation chain on DVE.

Note: indirect-DMA gathers use one [128,1]-index instruction per slot —
multi-index offset APs corrupt on HW through this runtime, and dma_gather
(int16 Q7 gather) hangs (its GPSIMD library never loads under axon/PJRT),
so ~2us/gather-instruction on the Pool engine is the binding constraint.
"""

from contextlib import ExitStack

import numpy as np

import concourse.bacc as bacc
import concourse.bass as bass
import concourse.mybir as mybir
import concourse.tile as tile
from concourse.bass_utils import run_bass_kernel_spmd
from concourse.masks import make_identity

# Problem shapes (hardcoded per contest contract).
P = 128
CD = 100          # char-embedding dim
L, R = 7, 10
K = L + R         # 17 context slots per query
KQ = K + 1        # + the query row itself
NCE = 200000      # ce table rows
V = 200000        # wvec table rows
WD = 300          # word-vector dim
B = 16384
N_CORES = 8
BC = B // N_CORES     # 2048 queries per core
NT = BC // P          # 16 tiles of 128 queries
CHUNK = 4             # tiles per phase chunk
OOB = 1 << 22         # stand-in index for -1; fails the DMA bounds check
N_POOL_K = 5          # wv accumulation steps offloaded to GPSIMD
WVP_BUFS = 3          # wv gather tile pool depth

F32 = mybir.dt.float32
I32 = mybir.dt.int32
Alu = mybir.AluOpType
Act = mybir.ActivationFunctionType
Ax = mybir.AxisListType


def _build_nc():
    nc = bacc.Bacc("TRN2", target_bir_lowering=False, debug=False,
                   num_devices=N_CORES)

    ce = nc.dram_tensor("ce_raw", [NCE, CD], F32, kind="ExternalInput")
    wv = nc.dram_tensor("wvec", [V, WD], F32, kind="ExternalInput")
    lW = nc.dram_tensor("lW", [CD, CD], F32, kind="ExternalInput")
    rW = nc.dram_tensor("rW", [CD, CD], F32, kind="ExternalInput")
    gw = nc.dram_tensor("gL_w", [2, K], F32, kind="ExternalInput")
    gb = nc.dram_tensor("gL_b", [2], F32, kind="ExternalInput")
    qidx = nc.dram_tensor("qidx", [BC], I32, kind="ExternalInput")
    lic = nc.dram_tensor("lixs_c", [BC, L], I32, kind="ExternalInput")
    ric = nc.dram_tensor("rixs_c", [BC, R], I32, kind="ExternalInput")
    liw = nc.dram_tensor("lixs_w", [BC, L], I32, kind="ExternalInput")
    riw = nc.dram_tensor("rixs_w", [BC, R], I32, kind="ExternalInput")
    out = nc.dram_tensor("out", [BC, WD], F32, kind="ExternalOutput")

    with tile.TileContext(nc) as tc, ExitStack() as ctx:
        consts = ctx.enter_context(tc.tile_pool(name="consts", bufs=1))
        stage = ctx.enter_context(tc.tile_pool(name="stage", bufs=1))
        cep = ctx.enter_context(tc.tile_pool(name="cep", bufs=4))
        wvp = ctx.enter_context(tc.tile_pool(name="wvp", bufs=WVP_BUFS))
        work = ctx.enter_context(tc.tile_pool(name="work", bufs=2))
        psum = ctx.enter_context(tc.tile_pool(name="psum", bufs=2, space="PSUM"))

        # ---------------- constants ----------------
        identity = consts.tile([P, P], F32)
        make_identity(nc, identity[:, :])

        # W^T for both sides packed as [100, 0:100]=lW^T, [100, 100:200]=rW^T
        wt_both = consts.tile([P, 2 * CD], F32)
        for side, wdram in enumerate((lW, rW)):
            wl = consts.tile([P, P], F32, name=f"wload{side}")
            nc.sync.dma_start(out=wl[0:CD, 0:CD], in_=wdram[:, :])
            wt_ps = psum.tile([P, P], F32, name=f"wt_ps{side}", tag="wt_ps")
            nc.tensor.transpose(
                out=wt_ps[0:CD, 0:CD], in_=wl[0:CD, 0:CD],
                identity=identity[0:CD, 0:CD])
            nc.vector.tensor_copy(
                out=wt_both[0:CD, side * CD:(side + 1) * CD],
                in_=wt_ps[0:CD, 0:CD])

        # gate weights replicated across partitions via PE outer product
        # (ones[128,1] @ row[1,36]); gwrep[:, j*K+k] = gL_w[j,k], cols 34:36=gL_b
        gwrow = consts.tile([1, 2 * K + 2], F32)
        nc.sync.dma_start(out=gwrow[0:1, 0:2 * K], in_=gw[:, :])
        nc.sync.dma_start(out=gwrow[0:1, 2 * K:2 * K + 2], in_=gb[:])
        ones1 = consts.tile([1, P], F32)
        nc.gpsimd.memset(ones1[:, :], 1.0)
        rep_ps = psum.tile([P, 2 * K + 2], F32, tag="rep_ps")
        nc.tensor.matmul(out=rep_ps[:, :], lhsT=ones1[0:1, :],
                         rhs=gwrow[0:1, :], start=True, stop=True)
        gwrep = consts.tile([P, 2 * K + 2], F32)
        nc.vector.tensor_copy(out=gwrep[:, :], in_=rep_ps[:, :])
        gbd = consts.tile([P, 1], F32)
        nc.vector.tensor_tensor(
            out=gbd[:, :], in0=gwrep[:, 2 * K + 1:2 * K + 2],
            in1=gwrep[:, 2 * K:2 * K + 1], op=Alu.subtract)

        # ---------------- index staging ----------------
        # cidx[p, t, 0] = qidx, [p, t, 1:8] = lixs_c, [p, t, 8:18] = rixs_c
        cidx = stage.tile([P, NT, KQ], I32)
        nc.sync.dma_start(out=cidx[:, :, 0],
                          in_=qidx[:].rearrange("(t p) -> p t", p=P))
        nc.sync.dma_start(out=cidx[:, :, 1:1 + L],
                          in_=lic[:, :].rearrange("(t p) k -> p t k", p=P))
        nc.sync.dma_start(out=cidx[:, :, 1 + L:KQ],
                          in_=ric[:, :].rearrange("(t p) k -> p t k", p=P))
        widx = stage.tile([P, NT, K], I32)
        nc.sync.dma_start(out=widx[:, :, 0:L],
                          in_=liw[:, :].rearrange("(t p) k -> p t k", p=P))
        nc.sync.dma_start(out=widx[:, :, L:K],
                          in_=riw[:, :].rearrange("(t p) k -> p t k", p=P))

        # clamped ce indices (pad -> row 0; masked later)
        ccl = stage.tile([P, NT, KQ], I32)
        nc.vector.tensor_scalar(out=ccl[:, :, :], in0=cidx[:, :, :],
                                scalar1=0, scalar2=None, op0=Alu.max)
        # wv indices: first WVP_BUFS tiles clamped (fully populate the pool
        # slots), later tiles map -1 -> OOB so the DMA bounds-check skips them
        wcl = stage.tile([P, NT, K], I32)
        nc.vector.tensor_scalar(out=wcl[:, :, :], in0=widx[:, :, :],
                                scalar1=0, scalar2=None, op0=Alu.max)
        wsk = stage.tile([P, NT, K], I32)
        nc.vector.tensor_scalar(out=wsk[:, :, :], in0=widx[:, :, :],
                                scalar1=0, scalar2=-OOB,
                                op0=Alu.min, op1=Alu.mult)
        nc.vector.tensor_tensor(out=wsk[:, :, :], in0=wsk[:, :, :],
                                in1=widx[:, :, :], op=Alu.add)

        # additive score mask: 0 for valid slots, -1e30 where index == -1
        maskf = stage.tile([P, NT, KQ], F32)
        nc.vector.tensor_copy(out=maskf[:, :, :], in_=cidx[:, :, :])
        maskt = stage.tile([P, NT, KQ], F32)
        nc.vector.tensor_scalar(out=maskt[:, :, :], in0=maskf[:, :, :],
                                scalar1=0.0, scalar2=1e30,
                                op0=Alu.min, op1=Alu.mult)

        # ---------------- staging buffers ----------------
        dotraw = stage.tile([P, NT, K], F32)
        cssq = stage.tile([P, NT, K], F32)
        qssq = stage.tile([P, NT], F32)
        expv = stage.tile([P, NT, K], F32)
        wall = stage.tile([P, NT, K], F32)

        def phase1_tile(t):
            ceg = cep.tile([P, KQ, CD], F32, name="ceg", tag="ceg")
            for s in range(KQ):
                nc.gpsimd.indirect_dma_start(
                    out=ceg[:, s, :], out_offset=None,
                    in_=ce[:, :],
                    in_offset=bass.IndirectOffsetOnAxis(
                        ap=ccl[:, t, s:s + 1], axis=0),
                )
            # transpose q rows: qt = ceg[:, 0, :]^T  -> [CD, P]
            qt_ps = psum.tile([P, P], F32, name="qt_ps", tag="qt_ps")
            nc.tensor.transpose(out=qt_ps[0:CD, :], in_=ceg[:, 0, :],
                                identity=identity[:, :])
            qt_sb = work.tile([P, P], F32, name="qt_sb", tag="qt_sb")
            nc.scalar.copy(out=qt_sb[0:CD, :], in_=qt_ps[0:CD, :])

            # u[q, 0:100] = lW @ q ; u[q, 100:200] = rW @ q
            u_ps = psum.tile([P, 2 * CD], F32, name="u_ps", tag="u_ps")
            nc.tensor.matmul(out=u_ps[:, :], lhsT=qt_sb[0:CD, :],
                             rhs=wt_both[0:CD, :], start=True, stop=True)

            # dot products: prod[p,k,c] = ctx[p,k,c] * u_side(k)[p,c]
            prod = work.tile([P, K, CD], F32, name="prod", tag="prod")
            nc.vector.tensor_tensor(
                out=prod[:, 0:L, :], in0=ceg[:, 1:1 + L, :],
                in1=u_ps[:, 0:CD].unsqueeze(1).broadcast_to((P, L, CD)),
                op=Alu.mult)
            nc.vector.tensor_tensor(
                out=prod[:, L:K, :], in0=ceg[:, 1 + L:KQ, :],
                in1=u_ps[:, CD:2 * CD].unsqueeze(1).broadcast_to((P, R, CD)),
                op=Alu.mult)
            nc.vector.tensor_reduce(out=dotraw[:, t, :], in_=prod[:, :, :],
                                    axis=Ax.X, op=Alu.add)

            # row sum-squares: ACT squares the whole gathered block (q+ctx),
            # DVE reduces each row
            sq = work.tile([P, KQ, CD], F32, name="sq", tag="sq")
            nc.scalar.activation(out=sq[:, :, :], in_=ceg[:, :, :],
                                 func=Act.Square)
            nc.vector.tensor_reduce(out=qssq[:, t:t + 1], in_=sq[:, 0, :],
                                    axis=Ax.X, op=Alu.add)
            nc.vector.tensor_reduce(out=cssq[:, t, :], in_=sq[:, 1:, :],
                                    axis=Ax.X, op=Alu.add)

        def phase2_chunk(t0, t1):
            n = t1 - t0
            ts = slice(t0, t1)
            # 1/max(||row||, 1e-12) for q and ctx rows
            sq_t = stage.tile([P, n], F32, name=f"sqt{t0}")
            nc.scalar.activation(out=sq_t[:, :], in_=qssq[:, ts],
                                 func=Act.Sqrt)
            nc.vector.tensor_scalar(out=sq_t[:, :], in0=sq_t[:, :],
                                    scalar1=1e-12, scalar2=None, op0=Alu.max)
            rq = stage.tile([P, n], F32, name=f"rq{t0}")
            nc.vector.reciprocal(out=rq[:, :], in_=sq_t[:, :])

            csq_t = stage.tile([P, n, K], F32, name=f"csqt{t0}")
            nc.scalar.activation(out=csq_t[:, :, :], in_=cssq[:, ts, :],
                                 func=Act.Sqrt)
            nc.vector.tensor_scalar(out=csq_t[:, :, :], in0=csq_t[:, :, :],
                                    scalar1=1e-12, scalar2=None, op0=Alu.max)
            rctx = stage.tile([P, n, K], F32, name=f"rctx{t0}")
            nc.vector.reciprocal(out=rctx[:, :, :], in_=csq_t[:, :, :])

            scr = stage.tile([P, n, K], F32, name=f"scr{t0}")
            nc.vector.tensor_tensor(out=scr[:, :, :], in0=dotraw[:, ts, :],
                                    in1=rctx[:, :, :], op=Alu.mult)
            nc.vector.tensor_tensor(
                out=scr[:, :, :], in0=scr[:, :, :],
                in1=rq[:, :].unsqueeze(2).broadcast_to((P, n, K)),
                op=Alu.mult)
            nc.vector.tensor_tensor(out=scr[:, :, :], in0=scr[:, :, :],
                                    in1=maskt[:, ts, 1:KQ], op=Alu.add)

            nc.scalar.activation(out=expv[:, ts, :], in_=scr[:, :, :],
                                 func=Act.Exp)

            sum_l = stage.tile([P, n], F32, name=f"suml{t0}")
            nc.vector.tensor_reduce(out=sum_l[:, :], in_=expv[:, ts, 0:L],
                                    axis=Ax.X, op=Alu.add)
            sum_r = stage.tile([P, n], F32, name=f"sumr{t0}")
            nc.vector.tensor_reduce(out=sum_r[:, :], in_=expv[:, ts, L:K],
                                    axis=Ax.X, op=Alu.add)
            rs_l = stage.tile([P, n], F32, name=f"rsl{t0}")
            nc.vector.reciprocal(out=rs_l[:, :], in_=sum_l[:, :])
            rs_r = stage.tile([P, n], F32, name=f"rsr{t0}")
            nc.vector.reciprocal(out=rs_r[:, :], in_=sum_r[:, :])

            # gate logit difference dz = (z1-z0) + (gb1-gb0), where
            # z_j = rs_l * sum_k exp_l[k] gw[j,k] + rs_r * sum_k exp_r[k] gw[j,..]
            d = {}
            gtmp_l = stage.tile([P, n, L], F32, name=f"gtl{t0}")
            gtmp_r = stage.tile([P, n, R], F32, name=f"gtr{t0}")
            for j in (0, 1):
                nc.vector.tensor_tensor(
                    out=gtmp_l[:, :, :], in0=expv[:, ts, 0:L],
                    in1=gwrep[:, j * K:j * K + L].unsqueeze(1)
                        .broadcast_to((P, n, L)),
                    op=Alu.mult)
                d[j, 'l'] = stage.tile([P, n], F32, name=f"d{j}l{t0}")
                nc.vector.tensor_reduce(out=d[j, 'l'][:, :],
                                        in_=gtmp_l[:, :, :],
                                        axis=Ax.X, op=Alu.add)
                nc.vector.tensor_tensor(
                    out=gtmp_r[:, :, :], in0=expv[:, ts, L:K],
                    in1=gwrep[:, j * K + L:(j + 1) * K].unsqueeze(1)
                        .broadcast_to((P, n, R)),
                    op=Alu.mult)
                d[j, 'r'] = stage.tile([P, n], F32, name=f"d{j}r{t0}")
                nc.vector.tensor_reduce(out=d[j, 'r'][:, :],
                                        in_=gtmp_r[:, :, :],
                                        axis=Ax.X, op=Alu.add)

            ddl = stage.tile([P, n], F32, name=f"ddl{t0}")
            nc.vector.tensor_tensor(out=ddl[:, :], in0=d[1, 'l'][:, :],
                                    in1=d[0, 'l'][:, :], op=Alu.subtract)
            ddr = stage.tile([P, n], F32, name=f"ddr{t0}")
            nc.vector.tensor_tensor(out=ddr[:, :], in0=d[1, 'r'][:, :],
                                    in1=d[0, 'r'][:, :], op=Alu.subtract)
            m1 = stage.tile([P, n], F32, name=f"m1{t0}")
            nc.vector.tensor_tensor(out=m1[:, :], in0=ddl[:, :],
                                    in1=rs_l[:, :], op=Alu.mult)
            m2 = stage.tile([P, n], F32, name=f"m2{t0}")
            nc.vector.tensor_tensor(out=m2[:, :], in0=ddr[:, :],
                                    in1=rs_r[:, :], op=Alu.mult)
            dz = stage.tile([P, n], F32, name=f"dz{t0}")
            nc.vector.tensor_tensor(out=dz[:, :], in0=m1[:, :], in1=m2[:, :],
                                    op=Alu.add)
            nc.vector.tensor_scalar(out=dz[:, :], in0=dz[:, :],
                                    scalar1=gbd[:, 0:1], scalar2=None,
                                    op0=Alu.add)

            e1 = stage.tile([P, n], F32, name=f"e1{t0}")
            nc.scalar.activation(out=e1[:, :], in_=dz[:, :], func=Act.Exp)
            den = stage.tile([P, n], F32, name=f"den{t0}")
            nc.vector.tensor_scalar(out=den[:, :], in0=e1[:, :], scalar1=1.0,
                                    scalar2=None, op0=Alu.add)
            rden = stage.tile([P, n], F32, name=f"rden{t0}")
            nc.vector.reciprocal(out=rden[:, :], in_=den[:, :])

            # c_l = g0*rs_l = rs_l/(1+e1); c_r = g1*rs_r = rs_r*e1/(1+e1)
            c_l = stage.tile([P, n], F32, name=f"cl{t0}")
            nc.vector.tensor_tensor(out=c_l[:, :], in0=rs_l[:, :],
                                    in1=rden[:, :], op=Alu.mult)
            c_r = stage.tile([P, n], F32, name=f"cr{t0}")
            nc.vector.tensor_tensor(out=c_r[:, :], in0=rs_r[:, :],
                                    in1=rden[:, :], op=Alu.mult)
            nc.vector.tensor_tensor(out=c_r[:, :], in0=c_r[:, :],
                                    in1=e1[:, :], op=Alu.mult)

            # final per-slot weights
            nc.vector.tensor_tensor(
                out=wall[:, ts, 0:L], in0=expv[:, ts, 0:L],
                in1=c_l[:, :].unsqueeze(2).broadcast_to((P, n, L)),
                op=Alu.mult)
            nc.vector.tensor_tensor(
                out=wall[:, ts, L:K], in0=expv[:, ts, L:K],
                in1=c_r[:, :].unsqueeze(2).broadcast_to((P, n, R)),
                op=Alu.mult)

        def phase3_tile(t):
            wvg = wvp.tile([P, K, WD], F32, name="wvg", tag="wvg")
            for s in range(K):
                if t < WVP_BUFS:
                    # clamped indices fully populate the pool slot
                    nc.gpsimd.indirect_dma_start(
                        out=wvg[:, s, :], out_offset=None,
                        in_=wv[:, :],
                        in_offset=bass.IndirectOffsetOnAxis(
                            ap=wcl[:, t, s:s + 1], axis=0),
                    )
                else:
                    # pad slots carry OOB indices -> descriptor skipped; the
                    # stale slot data is finite and weighted by exactly 0
                    nc.gpsimd.indirect_dma_start(
                        out=wvg[:, s, :], out_offset=None,
                        in_=wv[:, :],
                        in_offset=bass.IndirectOffsetOnAxis(
                            ap=wsk[:, t, s:s + 1], axis=0),
                        bounds_check=V - 1, oob_is_err=False,
                    )
            acc_a = work.tile([P, WD], F32, name="acc_a", tag="acc_a")
            acc_b = work.tile([P, WD], F32, name="acc_b", tag="acc_b")
            nc.vector.tensor_scalar(out=acc_a[:, :], in0=wvg[:, 0, :],
                                    scalar1=wall[:, t, 0:1], scalar2=None,
                                    op0=Alu.mult)
            for k in range(1, K):
                src, dst = (acc_a, acc_b) if k % 2 == 1 else (acc_b, acc_a)
                nc.vector.scalar_tensor_tensor(
                    out=dst[:, :], in0=wvg[:, k, :],
                    scalar=wall[:, t, k:k + 1], in1=src[:, :],
                    op0=Alu.mult, op1=Alu.add)
            res = acc_a if (K - 1) % 2 == 0 else acc_b
            nc.sync.dma_start(out=out[t * P:(t + 1) * P, :], in_=res[:, :])

        for c0 in range(0, NT, CHUNK):
            for t in range(c0, c0 + CHUNK):
                phase1_tile(t)
            phase2_chunk(c0, c0 + CHUNK)
            for t in range(c0, c0 + CHUNK):
                phase3_tile(t)

    nc.compile()
    return nc


_NC_CACHE = None


def _get_nc():
    global _NC_CACHE
    if _NC_CACHE is None:
        _NC_CACHE = _build_nc()
    return _NC_CACHE


def kernel(**inputs):
    inp = {k: np.asarray(v) for k, v in inputs.items()}
    nc = _get_nc()
    shared = {k: inp[k] for k in ("ce_raw", "wvec", "lW", "rW", "gL_w", "gL_b")}
    in_maps = []
    for c in range(N_CORES):
        sl = slice(c * BC, (c + 1) * BC)
        m = dict(shared)
        for name in ("qidx", "lixs_c", "rixs_c", "lixs_w", "rixs_w"):
            m[name] = np.ascontiguousarray(inp[name][sl])
        in_maps.append(m)
    res = run_bass_kernel_spmd(nc, in_maps, list(range(N_CORES)))
    return np.concatenate([res.results[c]["out"] for c in range(N_CORES)],
                          axis=0)



# revision 2
# speedup vs baseline: 1.5104x; 1.5104x over previous
"""Trainium2 Bass kernel for nn_Char_30322469110372 (retrieval_knn).

Reference computation (per query b):
  ce   = row-normalized ce_raw (+ zero pad row for index -1)
  q    = ce[qidx[b]]
  for side in (l, r):
    u_side      = W_side @ q                     # [C]
    score[k]    = ce[ixs_c[b,k]] . u_side        # masked to -1e30 where ixs==-1
    attn        = softmax(score)
    emb_side    = sum_k attn[k] * wvec[ixs_w[b,k]]
  gate = softmax([attn_l, attn_r] @ gL_w.T + gL_b)
  out  = gate[0]*emb_l + gate[1]*emb_r

Sharding: data-parallel over B across 8 cores; ce/wvec tables replicated.

Key cost fact (TimelineSim + HW): every indirect-DMA gather instruction
carries exactly ONE index per partition (multi-index offset APs corrupt on
this runtime -- verified by probe) and costs ~1037ns of Pool-engine SWDGE
descriptor-generation time (994ns fixed + 128*0.34ns). The baseline's 560
gathers/core (18 ce + 17 wv slots x 16 tiles) == ~581us Pool-serial wall.

This version cuts the gather count by exploiting the trailing-pad structure:
llen ~ U{1..7}, rlen ~ U{1..10}, so on average only 10.5/18 ce slots and
9.5/17 wv slots are valid. kernel() groups queries by (llen, rlen) into 16
tiles-of-1024 (8 cores x 128 partitions) chosen by a small transportation-
feasibility optimizer so that each tile's (maxL, maxR) corner is tight, then
compiles a kernel that only gathers slots < (maxL, maxR) per tile. All
queries keep exact semantics: per-query pads inside a gathered range get
score -1e30 -> softmax weight exactly 0 (clamped index gathers row 0, which
is then weighted by exactly 0). Structurally skipped lanes read memset
staging (finite) and are masked to -1e30 the same way.

Device algorithm per core (2048 queries, 16 tiles of 128, chunks of 4):
  - normalization folded into scores: score = (ctx_raw.u_raw) * rctx * rq.
  - scores bounded (|score| <= ~1.2) so softmax needs no max-shift.
  - per tile: indirect-DMA gather of 1+maxL+maxR ce rows/query; PE transposes
    q and computes u = [qT]^T @ [lW^T | rW^T]; DVE dot products over valid
    lanes; ACT squares; DVE row sum-squares.
  - per chunk: softmax + gate pipeline on [128, 4*17] staging buffers.
  - per tile: gather maxL+maxR wvec rows/query (pad slots clamped to row 0,
    weight exactly 0), then a scalar*tensor+tensor accumulation chain on DVE.
"""

from contextlib import ExitStack

import numpy as np

import concourse.bacc as bacc
import concourse.bass as bass
import concourse.mybir as mybir
import concourse.tile as tile
from concourse.bass_utils import run_bass_kernel_spmd
from concourse.masks import make_identity

# Problem shapes (hardcoded per contest contract).
P = 128
CD = 100          # char-embedding dim
L, R = 7, 10
K = L + R         # 17 context slots per query
KQ = K + 1        # + the query row itself
NCE = 200000      # ce table rows
V = 200000        # wvec table rows
WD = 300          # word-vector dim
B = 16384
N_CORES = 8
BC = B // N_CORES     # 2048 queries per core
NT = BC // P          # 16 tiles of 128 queries
STRIPE = P * N_CORES  # 1024 queries per tile-stripe across cores
CHUNK = 4             # tiles per phase chunk

F32 = mybir.dt.float32
I32 = mybir.dt.int32
Alu = mybir.AluOpType
Act = mybir.ActivationFunctionType
Ax = mybir.AxisListType


# --------------------------------------------------------------------------
# Host-side planner: group queries into 16 stripes of 1024 so each stripe's
# (maxL, maxR) corner is tight.  Cost per stripe = 2*(L+R)+1 gather instrs.
# --------------------------------------------------------------------------

def _feasible_flow(corners, buckets, cnt, cap):
    """Transportation feasibility: bucket (l,r) may go to any group whose
    corner dominates it; each group takes exactly `cap` queries. Returns the
    flow dict {(bucket_i, group_g): n} if feasible else None."""
    import collections
    nb, ng = len(buckets), len(corners)
    s, t = 0, nb + ng + 1
    capm = {}
    for i, b in enumerate(buckets):
        capm[(s, 1 + i)] = cnt[b]
    for g, (cl, cr) in enumerate(corners):
        capm[(1 + nb + g, t)] = cap
        for i, (l, r) in enumerate(buckets):
            if l <= cl and r <= cr:
                capm[(1 + i, 1 + nb + g)] = 1 << 30
    flow = collections.defaultdict(int)
    adj = collections.defaultdict(list)
    for (u, v) in capm:
        adj[u].append(v)
        adj[v].append(u)
    total = 0
    while True:
        par = {s: None}
        dq = collections.deque([s])
        while dq and t not in par:
            u = dq.popleft()
            for v in adj[u]:
                if v not in par and capm.get((u, v), 0) - flow[(u, v)] + flow[(v, u)] > 0:
                    par[v] = u
                    dq.append(v)
        if t not in par:
            break
        path = []
        v = t
        while par[v] is not None:
            path.append((par[v], v))
            v = par[v]
        aug = min(capm.get((u, v), 0) - flow[(u, v)] + flow[(v, u)] for u, v in path)
        for u, v in path:
            if flow[(v, u)] >= aug:
                flow[(v, u)] -= aug
            else:
                flow[(u, v)] += aug - flow[(v, u)]
                flow[(v, u)] = 0
        total += aug
    if total != sum(cnt.values()):
        return None
    out = {}
    for i in range(nb):
        for g in range(ng):
            f = flow.get((1 + i, 1 + nb + g), 0)
            if f > 0:
                out[(i, g)] = f
    return out


def _plan(llen, rlen):
    """Returns (corners[NT], qtile[NT] lists of 1024 query ids)."""
    import random
    cnt = {}
    for l, r in zip(llen.tolist(), rlen.tolist()):
        cnt[(l, r)] = cnt.get((l, r), 0) + 1
    buckets = sorted(cnt.keys())

    def feas(corners):
        return _feasible_flow(corners, buckets, cnt, STRIPE) is not None

    # start: contiguous fill of an r-major snake sort
    snake = np.where(rlen % 2 == 0, 1000 - llen, llen)
    order = np.lexsort((snake, rlen))
    corners = []
    for tt in range(NT):
        sl = order[tt * STRIPE:(tt + 1) * STRIPE]
        corners.append((int(llen[sl].max()), int(rlen[sl].max())))

    def cost(cs):
        return sum(cl + cr for cl, cr in cs)

    def descend(cs):
        cs = list(cs)
        improved = True
        while improved:
            improved = False
            for gi in range(NT):
                for d in ((-1, 0), (0, -1), (-1, -1), (-2, 0), (0, -2)):
                    cl, cr = cs[gi]
                    nl, nr = cl + d[0], cr + d[1]
                    if nl < 1 or nr < 1:
                        continue
                    cand = list(cs)
                    cand[gi] = (nl, nr)
                    if feas(cand):
                        cs = cand
                        improved = True
        return cs

    corners = descend(corners)
    rng = random.Random(0)
    moves = [(1, 0, -1, 0), (0, 1, 0, -1), (1, 0, 0, -1), (0, 1, -1, 0),
             (1, 1, -1, -1), (2, 0, -1, 0), (0, 2, 0, -1)]
    for _ in range(1200):
        gi, gj = rng.randrange(NT), rng.randrange(NT)
        if gi == gj:
            continue
        m = moves[rng.randrange(len(moves))]
        li, ri = corners[gi]
        lj, rj = corners[gj]
        nli, nri, nlj, nrj = li + m[0], ri + m[1], lj + m[2], rj + m[3]
        if not (1 <= nli <= L and 1 <= nri <= R and 1 <= nlj <= L and 1 <= nrj <= R):
            continue
        cand = list(corners)
        cand[gi] = (nli, nri)
        cand[gj] = (nlj, nrj)
        if cost(cand) <= cost(corners) and feas(cand):
            cand = descend(cand)
            if cost(cand) < cost(corners):
                corners = cand

    flow = _feasible_flow(corners, buckets, cnt, STRIPE)
    assert flow is not None

    # materialize query ids per bucket, then deal them out per the flow
    by_bucket = {b: [] for b in buckets}
    for qi, (l, r) in enumerate(zip(llen.tolist(), rlen.tolist())):
        by_bucket[(l, r)].append(qi)
    qtile = [[] for _ in range(NT)]
    used = {b: 0 for b in buckets}
    for (i, g), n in sorted(flow.items()):
        b = buckets[i]
        qtile[g].extend(by_bucket[b][used[b]:used[b] + n])
        used[b] += n
    for g in range(NT):
        assert len(qtile[g]) == STRIPE
    # heaviest tiles first (marginally better pipeline overlap)
    order2 = sorted(range(NT), key=lambda g: -(corners[g][0] + corners[g][1]))
    corners = [corners[g] for g in order2]
    qtile = [qtile[g] for g in order2]
    return corners, qtile


# --------------------------------------------------------------------------
# Device kernel, parameterized by the per-tile (maxL, maxR) corners.
# --------------------------------------------------------------------------

def _build_nc(corners):
    nc = bacc.Bacc("TRN2", target_bir_lowering=False, debug=False,
                   num_devices=N_CORES)

    ce = nc.dram_tensor("ce_raw", [NCE, CD], F32, kind="ExternalInput")
    wv = nc.dram_tensor("wvec", [V, WD], F32, kind="ExternalInput")
    lW = nc.dram_tensor("lW", [CD, CD], F32, kind="ExternalInput")
    rW = nc.dram_tensor("rW", [CD, CD], F32, kind="ExternalInput")
    gw = nc.dram_tensor("gL_w", [2, K], F32, kind="ExternalInput")
    gb = nc.dram_tensor("gL_b", [2], F32, kind="ExternalInput")
    qidx = nc.dram_tensor("qidx", [BC], I32, kind="ExternalInput")
    lic = nc.dram_tensor("lixs_c", [BC, L], I32, kind="ExternalInput")
    ric = nc.dram_tensor("rixs_c", [BC, R], I32, kind="ExternalInput")
    liw = nc.dram_tensor("lixs_w", [BC, L], I32, kind="ExternalInput")
    riw = nc.dram_tensor("rixs_w", [BC, R], I32, kind="ExternalInput")
    out = nc.dram_tensor("out", [BC, WD], F32, kind="ExternalOutput")

    with tile.TileContext(nc) as tc, ExitStack() as ctx:
        consts = ctx.enter_context(tc.tile_pool(name="consts", bufs=1))
        stage = ctx.enter_context(tc.tile_pool(name="stage", bufs=1))
        cep = ctx.enter_context(tc.tile_pool(name="cep", bufs=4))
        wvp = ctx.enter_context(tc.tile_pool(name="wvp", bufs=3))
        work = ctx.enter_context(tc.tile_pool(name="work", bufs=2))
        psum = ctx.enter_context(tc.tile_pool(name="psum", bufs=2, space="PSUM"))

        # ---------------- constants ----------------
        identity = consts.tile([P, P], F32)
        make_identity(nc, identity[:, :])

        # W^T for both sides packed as [100, 0:100]=lW^T, [100, 100:200]=rW^T
        wt_both = consts.tile([P, 2 * CD], F32)
        for side, wdram in enumerate((lW, rW)):
            wl = consts.tile([P, P], F32, name=f"wload{side}")
            nc.sync.dma_start(out=wl[0:CD, 0:CD], in_=wdram[:, :])
            wt_ps = psum.tile([P, P], F32, name=f"wt_ps{side}", tag="wt_ps")
            nc.tensor.transpose(
                out=wt_ps[0:CD, 0:CD], in_=wl[0:CD, 0:CD],
                identity=identity[0:CD, 0:CD])
            nc.vector.tensor_copy(
                out=wt_both[0:CD, side * CD:(side + 1) * CD],
                in_=wt_ps[0:CD, 0:CD])

        # gate weights replicated across partitions via PE outer product
        # (ones[128,1] @ row[1,36]); gwrep[:, j*K+k] = gL_w[j,k], cols 34:36=gL_b
        gwrow = consts.tile([1, 2 * K + 2], F32)
        nc.sync.dma_start(out=gwrow[0:1, 0:2 * K], in_=gw[:, :])
        nc.sync.dma_start(out=gwrow[0:1, 2 * K:2 * K + 2], in_=gb[:])
        ones1 = consts.tile([1, P], F32)
        nc.gpsimd.memset(ones1[:, :], 1.0)
        rep_ps = psum.tile([P, 2 * K + 2], F32, tag="rep_ps")
        nc.tensor.matmul(out=rep_ps[:, :], lhsT=ones1[0:1, :],
                         rhs=gwrow[0:1, :], start=True, stop=True)
        gwrep = consts.tile([P, 2 * K + 2], F32)
        nc.vector.tensor_copy(out=gwrep[:, :], in_=rep_ps[:, :])
        gbd = consts.tile([P, 1], F32)
        nc.vector.tensor_tensor(
            out=gbd[:, :], in0=gwrep[:, 2 * K + 1:2 * K + 2],
            in1=gwrep[:, 2 * K:2 * K + 1], op=Alu.subtract)

        # ---------------- index staging ----------------
        # cidx[p, t, 0] = qidx, [p, t, 1:8] = lixs_c, [p, t, 8:18] = rixs_c
        cidx = stage.tile([P, NT, KQ], I32)
        nc.sync.dma_start(out=cidx[:, :, 0],
                          in_=qidx[:].rearrange("(t p) -> p t", p=P))
        nc.sync.dma_start(out=cidx[:, :, 1:1 + L],
                          in_=lic[:, :].rearrange("(t p) k -> p t k", p=P))
        nc.sync.dma_start(out=cidx[:, :, 1 + L:KQ],
                          in_=ric[:, :].rearrange("(t p) k -> p t k", p=P))
        widx = stage.tile([P, NT, K], I32)
        nc.sync.dma_start(out=widx[:, :, 0:L],
                          in_=liw[:, :].rearrange("(t p) k -> p t k", p=P))
        nc.sync.dma_start(out=widx[:, :, L:K],
                          in_=riw[:, :].rearrange("(t p) k -> p t k", p=P))

        # clamped indices (pad -> row 0; weight is exactly 0 for pads)
        ccl = stage.tile([P, NT, KQ], I32)
        nc.vector.tensor_scalar(out=ccl[:, :, :], in0=cidx[:, :, :],
                                scalar1=0, scalar2=None, op0=Alu.max)
        wcl = stage.tile([P, NT, K], I32)
        nc.vector.tensor_scalar(out=wcl[:, :, :], in0=widx[:, :, :],
                                scalar1=0, scalar2=None, op0=Alu.max)

        # additive score mask: 0 for valid slots, -1e30 where index == -1
        maskf = stage.tile([P, NT, KQ], F32)
        nc.vector.tensor_copy(out=maskf[:, :, :], in_=cidx[:, :, :])
        maskt = stage.tile([P, NT, KQ], F32)
        nc.vector.tensor_scalar(out=maskt[:, :, :], in0=maskf[:, :, :],
                                scalar1=0.0, scalar2=1e30,
                                op0=Alu.min, op1=Alu.mult)

        # ---------------- staging buffers ----------------
        # memset to 1.0 so structurally-skipped lanes stay finite; their
        # scores get -1e30 from maskt (their cidx is -1 for every query in
        # the tile by construction) and exp() underflows to exactly 0.
        dotraw = stage.tile([P, NT, K], F32)
        nc.gpsimd.memset(dotraw[:, :, :], 1.0)
        cssq = stage.tile([P, NT, K], F32)
        nc.gpsimd.memset(cssq[:, :, :], 1.0)
        qssq = stage.tile([P, NT], F32)
        expv = stage.tile([P, NT, K], F32)
        wall = stage.tile([P, NT, K], F32)

        def phase1_tile(t):
            mL, mR = corners[t]
            ce_slots = [0] + list(range(1, 1 + mL)) + list(range(1 + L, 1 + L + mR))
            ceg = cep.tile([P, KQ, CD], F32, name="ceg", tag="ceg")
            for s in ce_slots:
                nc.gpsimd.indirect_dma_start(
                    out=ceg[:, s, :], out_offset=None,
                    in_=ce[:, :],
                    in_offset=bass.IndirectOffsetOnAxis(
                        ap=ccl[:, t, s:s + 1], axis=0),
                )
            # transpose q rows: qt = ceg[:, 0, :]^T  -> [CD, P]
            qt_ps = psum.tile([P, P], F32, name="qt_ps", tag="qt_ps")
            nc.tensor.transpose(out=qt_ps[0:CD, :], in_=ceg[:, 0, :],
                                identity=identity[:, :])
            qt_sb = work.tile([P, P], F32, name="qt_sb", tag="qt_sb")
            nc.scalar.copy(out=qt_sb[0:CD, :], in_=qt_ps[0:CD, :])

            # u[q, 0:100] = lW @ q ; u[q, 100:200] = rW @ q
            u_ps = psum.tile([P, 2 * CD], F32, name="u_ps", tag="u_ps")
            nc.tensor.matmul(out=u_ps[:, :], lhsT=qt_sb[0:CD, :],
                             rhs=wt_both[0:CD, :], start=True, stop=True)

            # dot products over valid lanes only
            prod = work.tile([P, K, CD], F32, name="prod", tag="prod")
            nc.vector.tensor_tensor(
                out=prod[:, 0:mL, :], in0=ceg[:, 1:1 + mL, :],
                in1=u_ps[:, 0:CD].unsqueeze(1).broadcast_to((P, mL, CD)),
                op=Alu.mult)
            nc.vector.tensor_tensor(
                out=prod[:, L:L + mR, :], in0=ceg[:, 1 + L:1 + L + mR, :],
                in1=u_ps[:, CD:2 * CD].unsqueeze(1).broadcast_to((P, mR, CD)),
                op=Alu.mult)
            nc.vector.tensor_reduce(out=dotraw[:, t, 0:mL],
                                    in_=prod[:, 0:mL, :],
                                    axis=Ax.X, op=Alu.add)
            nc.vector.tensor_reduce(out=dotraw[:, t, L:L + mR],
                                    in_=prod[:, L:L + mR, :],
                                    axis=Ax.X, op=Alu.add)

            # row sum-squares over gathered lanes (q + valid ctx)
            sq = work.tile([P, KQ, CD], F32, name="sq", tag="sq")
            nc.scalar.activation(out=sq[:, 0:1 + mL, :],
                                 in_=ceg[:, 0:1 + mL, :], func=Act.Square)
            nc.scalar.activation(out=sq[:, 1 + L:1 + L + mR, :],
                                 in_=ceg[:, 1 + L:1 + L + mR, :],
                                 func=Act.Square)
            nc.vector.tensor_reduce(out=qssq[:, t:t + 1], in_=sq[:, 0, :],
                                    axis=Ax.X, op=Alu.add)
            nc.vector.tensor_reduce(out=cssq[:, t, 0:mL],
                                    in_=sq[:, 1:1 + mL, :],
                                    axis=Ax.X, op=Alu.add)
            nc.vector.tensor_reduce(out=cssq[:, t, L:L + mR],
                                    in_=sq[:, 1 + L:1 + L + mR, :],
                                    axis=Ax.X, op=Alu.add)

        def phase2_chunk(t0, t1):
            n = t1 - t0
            ts = slice(t0, t1)
            # 1/max(||row||, 1e-12) for q and ctx rows
            sq_t = stage.tile([P, n], F32, name=f"sqt{t0}")
            nc.scalar.activation(out=sq_t[:, :], in_=qssq[:, ts],
                                 func=Act.Sqrt)
            nc.vector.tensor_scalar(out=sq_t[:, :], in0=sq_t[:, :],
                                    scalar1=1e-12, scalar2=None, op0=Alu.max)
            rq = stage.tile([P, n], F32, name=f"rq{t0}")
            nc.vector.reciprocal(out=rq[:, :], in_=sq_t[:, :])

            csq_t = stage.tile([P, n, K], F32, name=f"csqt{t0}")
            nc.scalar.activation(out=csq_t[:, :, :], in_=cssq[:, ts, :],
                                 func=Act.Sqrt)
            nc.vector.tensor_scalar(out=csq_t[:, :, :], in0=csq_t[:, :, :],
                                    scalar1=1e-12, scalar2=None, op0=Alu.max)
            rctx = stage.tile([P, n, K], F32, name=f"rctx{t0}")
            nc.vector.reciprocal(out=rctx[:, :, :], in_=csq_t[:, :, :])

            scr = stage.tile([P, n, K], F32, name=f"scr{t0}")
            nc.vector.tensor_tensor(out=scr[:, :, :], in0=dotraw[:, ts, :],
                                    in1=rctx[:, :, :], op=Alu.mult)
            nc.vector.tensor_tensor(
                out=scr[:, :, :], in0=scr[:, :, :],
                in1=rq[:, :].unsqueeze(2).broadcast_to((P, n, K)),
                op=Alu.mult)
            nc.vector.tensor_tensor(out=scr[:, :, :], in0=scr[:, :, :],
                                    in1=maskt[:, ts, 1:KQ], op=Alu.add)

            nc.scalar.activation(out=expv[:, ts, :], in_=scr[:, :, :],
                                 func=Act.Exp)

            sum_l = stage.tile([P, n], F32, name=f"suml{t0}")
            nc.vector.tensor_reduce(out=sum_l[:, :], in_=expv[:, ts, 0:L],
                                    axis=Ax.X, op=Alu.add)
            sum_r = stage.tile([P, n], F32, name=f"sumr{t0}")
            nc.vector.tensor_reduce(out=sum_r[:, :], in_=expv[:, ts, L:K],
                                    axis=Ax.X, op=Alu.add)
            rs_l = stage.tile([P, n], F32, name=f"rsl{t0}")
            nc.vector.reciprocal(out=rs_l[:, :], in_=sum_l[:, :])
            rs_r = stage.tile([P, n], F32, name=f"rsr{t0}")
            nc.vector.reciprocal(out=rs_r[:, :], in_=sum_r[:, :])

            # gate logit difference dz = (z1-z0) + (gb1-gb0), where
            # z_j = rs_l * sum_k exp_l[k] gw[j,k] + rs_r * sum_k exp_r[k] gw[j,..]
            d = {}
            gtmp_l = stage.tile([P, n, L], F32, name=f"gtl{t0}")
            gtmp_r = stage.tile([P, n, R], F32, name=f"gtr{t0}")
            for j in (0, 1):
                nc.vector.tensor_tensor(
                    out=gtmp_l[:, :, :], in0=expv[:, ts, 0:L],
                    in1=gwrep[:, j * K:j * K + L].unsqueeze(1)
                        .broadcast_to((P, n, L)),
                    op=Alu.mult)
                d[j, 'l'] = stage.tile([P, n], F32, name=f"d{j}l{t0}")
                nc.vector.tensor_reduce(out=d[j, 'l'][:, :],
                                        in_=gtmp_l[:, :, :],
                                        axis=Ax.X, op=Alu.add)
                nc.vector.tensor_tensor(
                    out=gtmp_r[:, :, :], in0=expv[:, ts, L:K],
                    in1=gwrep[:, j * K + L:(j + 1) * K].unsqueeze(1)
                        .broadcast_to((P, n, R)),
                    op=Alu.mult)
                d[j, 'r'] = stage.tile([P, n], F32, name=f"d{j}r{t0}")
                nc.vector.tensor_reduce(out=d[j, 'r'][:, :],
                                        in_=gtmp_r[:, :, :],
                                        axis=Ax.X, op=Alu.add)

            ddl = stage.tile([P, n], F32, name=f"ddl{t0}")
            nc.vector.tensor_tensor(out=ddl[:, :], in0=d[1, 'l'][:, :],
                                    in1=d[0, 'l'][:, :], op=Alu.subtract)
            ddr = stage.tile([P, n], F32, name=f"ddr{t0}")
            nc.vector.tensor_tensor(out=ddr[:, :], in0=d[1, 'r'][:, :],
                                    in1=d[0, 'r'][:, :], op=Alu.subtract)
            m1 = stage.tile([P, n], F32, name=f"m1{t0}")
            nc.vector.tensor_tensor(out=m1[:, :], in0=ddl[:, :],
                                    in1=rs_l[:, :], op=Alu.mult)
            m2 = stage.tile([P, n], F32, name=f"m2{t0}")
            nc.vector.tensor_tensor(out=m2[:, :], in0=ddr[:, :],
                                    in1=rs_r[:, :], op=Alu.mult)
            dz = stage.tile([P, n], F32, name=f"dz{t0}")
            nc.vector.tensor_tensor(out=dz[:, :], in0=m1[:, :], in1=m2[:, :],
                                    op=Alu.add)
            nc.vector.tensor_scalar(out=dz[:, :], in0=dz[:, :],
                                    scalar1=gbd[:, 0:1], scalar2=None,
                                    op0=Alu.add)

            e1 = stage.tile([P, n], F32, name=f"e1{t0}")
            nc.scalar.activation(out=e1[:, :], in_=dz[:, :], func=Act.Exp)
            den = stage.tile([P, n], F32, name=f"den{t0}")
            nc.vector.tensor_scalar(out=den[:, :], in0=e1[:, :], scalar1=1.0,
                                    scalar2=None, op0=Alu.add)
            rden = stage.tile([P, n], F32, name=f"rden{t0}")
            nc.vector.reciprocal(out=rden[:, :], in_=den[:, :])

            # c_l = g0*rs_l = rs_l/(1+e1); c_r = g1*rs_r = rs_r*e1/(1+e1)
            c_l = stage.tile([P, n], F32, name=f"cl{t0}")
            nc.vector.tensor_tensor(out=c_l[:, :], in0=rs_l[:, :],
                                    in1=rden[:, :], op=Alu.mult)
            c_r = stage.tile([P, n], F32, name=f"cr{t0}")
            nc.vector.tensor_tensor(out=c_r[:, :], in0=rs_r[:, :],
                                    in1=rden[:, :], op=Alu.mult)
            nc.vector.tensor_tensor(out=c_r[:, :], in0=c_r[:, :],
                                    in1=e1[:, :], op=Alu.mult)

            # final per-slot weights
            nc.vector.tensor_tensor(
                out=wall[:, ts, 0:L], in0=expv[:, ts, 0:L],
                in1=c_l[:, :].unsqueeze(2).broadcast_to((P, n, L)),
                op=Alu.mult)
            nc.vector.tensor_tensor(
                out=wall[:, ts, L:K], in0=expv[:, ts, L:K],
                in1=c_r[:, :].unsqueeze(2).broadcast_to((P, n, R)),
                op=Alu.mult)

        def phase3_tile(t):
            mL, mR = corners[t]
            wv_slots = list(range(0, mL)) + list(range(L, L + mR))
            wvg = wvp.tile([P, K, WD], F32, name="wvg", tag="wvg")
            for s in wv_slots:
                nc.gpsimd.indirect_dma_start(
                    out=wvg[:, s, :], out_offset=None,
                    in_=wv[:, :],
                    in_offset=bass.IndirectOffsetOnAxis(
                        ap=wcl[:, t, s:s + 1], axis=0),
                )
            acc_a = work.tile([P, WD], F32, name="acc_a", tag="acc_a")
            acc_b = work.tile([P, WD], F32, name="acc_b", tag="acc_b")
            s0 = wv_slots[0]
            nc.vector.tensor_scalar(out=acc_a[:, :], in0=wvg[:, s0, :],
                                    scalar1=wall[:, t, s0:s0 + 1],
                                    scalar2=None, op0=Alu.mult)
            src, dst = acc_a, acc_b
            for s in wv_slots[1:]:
                nc.vector.scalar_tensor_tensor(
                    out=dst[:, :], in0=wvg[:, s, :],
                    scalar=wall[:, t, s:s + 1], in1=src[:, :],
                    op0=Alu.mult, op1=Alu.add)
                src, dst = dst, src
            nc.sync.dma_start(out=out[t * P:(t + 1) * P, :], in_=src[:, :])

        for c0 in range(0, NT, CHUNK):
            for t in range(c0, c0 + CHUNK):
                phase1_tile(t)
            phase2_chunk(c0, c0 + CHUNK)
            for t in range(c0, c0 + CHUNK):
                phase3_tile(t)

    nc.compile()
    return nc


_NC_CACHE: dict = {}


def _get_nc(corners):
    key = tuple(corners)
    if key not in _NC_CACHE:
        _NC_CACHE[key] = _build_nc(list(corners))
    return _NC_CACHE[key]


def kernel(**inputs):
    inp = {k: np.asarray(v) for k, v in inputs.items()}
    llen = (inp["lixs_c"] != -1).sum(1).astype(np.int64)
    rlen = (inp["rixs_c"] != -1).sum(1).astype(np.int64)
    corners, qtile = _plan(llen, rlen)
    nc = _get_nc(corners)

    # per-core query permutation: core c, tile t, partition p <- qtile[t][c*P+p]
    percore = []
    for c in range(N_CORES):
        ids = np.empty(BC, dtype=np.int64)
        for t in range(NT):
            ids[t * P:(t + 1) * P] = qtile[t][c * P:(c + 1) * P]
        percore.append(ids)

    shared = {k: inp[k] for k in ("ce_raw", "wvec", "lW", "rW", "gL_w", "gL_b")}
    in_maps = []
    for c in range(N_CORES):
        m = dict(shared)
        for name in ("qidx", "lixs_c", "rixs_c", "lixs_w", "rixs_w"):
            m[name] = np.ascontiguousarray(inp[name][percore[c]])
        in_maps.append(m)
    res = run_bass_kernel_spmd(nc, in_maps, list(range(N_CORES)))
    full = np.empty((B, WD), dtype=np.float32)
    for c in range(N_CORES):
        full[percore[c]] = res.results[c]["out"]
    return full
